# revision 9
# baseline (speedup 1.0000x reference)
"""Multi-head attention (B=2, S=2048, D=1024, 16 heads x 64) on 8 NeuronCores.

Sharding: batch x head-group data/tensor parallel. Core c handles batch
c//4 and heads [4*(c%4), 4*(c%4)+4). Wq/Wk/Wv are column-sliced per head
group, Wo row-sliced; each core emits a partial [S, D] output (bf16) and
the host sums the 4 partials per batch and adds bo + bv@Wo (the V bias
commutes through attention since softmax rows sum to 1, so it is folded
into the final bias host-side).

v2 over the first working kernel (243.6us):
  1. Pair-packed Q^T/K^T: heads 2m / 2m+1 live in partitions 0:64 /
     64:128 of one [128, S] tile. The two heads' score matmuls (K=64
     contraction each) then run CONCURRENTLY in disjoint 32-row groups
     of the PE array (tile_position auto-derived from base_partition),
     halving scores PE time.
  2. exp split across ACT and a custom DVE op pair: P1 = deg-4 Horner
     q ~= exp(s/32)/c0 (8 ALU stages; coeffs via s0/s1/imm2 + C3 spill;
     constant c0 normalized away - softmax is scale-invariant, and the
     ACT path matches the scale via exp bias = -4*ln(c0)); P2 = q^4 via
     two squarings. ~0.01% poly error. ACT alone was the 136us phase
     bottleneck; split, both engines stay under the PE's attention time.
  3. V projected directly into natural [seq, dh] layout (stationary =
     x^T seq-chunk, moving = Wv), killing the PE transposes. Per-head
     128-col stationary blocks hold [V|1|0] (even head) / [1|0|V] (odd
     head) so attnV emits O^T rows partition-aligned for both heads and
     row 64/0 of the accumulator is the softmax denominator.
  4. Garbage-operand warmup matmuls at t=0 (no DMA deps) open the HAM
     clock gate (~3.4us busy window) before the real projections start;
     the whole PE stream is then gap-free so the gate never re-closes.
  5. Output projection as a short tail phase (PSUM too tight to drip it
     through attention); partials DMA'd as bf16 to halve the tail DMA.
"""

import sys

if "/opt/trn_rl_repo" not in sys.path:
    sys.path.insert(0, "/opt/trn_rl_repo")

import numpy as np

import concourse.bacc as bacc
import concourse.mybir as mybir
import concourse.tile as tile
from concourse.bass_utils import run_bass_kernel_spmd

# ---- custom DVE exp ops (registered at import; appended to the op table
# so existing rows keep their opcodes) --------------------------------------
import concourse.dve_ops as dve_ops
from concourse.dve_ops import DveOp
from concourse.dve_spec import (
    C0, C1, C2, C3, One, Src0, Spec, sq, _spill_c3_to_src1, _has_src1, lower,
)
from concourse.dve_uop import DveOpSpec
from concourse.dve_table_gen import dve_ver_for

# deg-4 weighted-minimax fit of exp(s/32), normalized so the constant
# term is 1 (true constant c0 below; softmax cancels it, ACT compensates)
D4, D3, D2, D1 = (4.117192751426077e-08, 5.305616453292819e-06,
                  0.0004883285606669467, 0.031224638167200523)
C0_NORM = 0.9999729446821336
ACT_BIAS = float(-4.0 * np.log(C0_NORM))  # exp(s/8 + bias) = exp(s/8)/c0^4


def _p1_ref(in0, in1, s0, s1, imm2):
    x = in0.astype(np.float32)
    return ((((s0 * x + s1) * x + imm2) * x + in1) * x) + 1.0


def _p2_ref(in0, in1, s0, s1, imm2):
    x = in0.astype(np.float32)
    return (x * x) * (x * x)


def _register(name, body, ref):
    for op in dve_ops.OPS:
        if op.name == name:
            return op
    spec = Spec(body=body, reference=ref)
    row = dve_ops._CUSTOM_DVE_ROW_BASE + len(dve_ops.OPS)
    ver = dve_ver_for("TRN2")
    sha = DveOpSpec(
        name=name, opcode=row, uops=lower(spec, ver=ver),
        rd1_en=_has_src1(spec),
    ).sha(ver)
    op = DveOp(name, spec, subdim=False, uops_sha={ver: sha})
    dve_ops.OPS.append(op)
    dve_ops._SUB_OPCODE_FOR_NAME[name] = row
    dve_ops.CUSTOM_DVE_SPECS[name] = spec
    return op


EXP_P1 = _register(
    "EXP_P1_ANT",
    _spill_c3_to_src1(((((C0 * Src0 + C1) * Src0 + C2) * Src0 + C3) * Src0) + One),
    _p1_ref,
)
EXP_SQSQ = _register("EXP_SQSQ_ANT", sq(sq(Src0)), _p2_ref)

F32 = mybir.dt.float32
F16 = mybir.dt.float16
BF16 = mybir.dt.bfloat16
NPDT = np.float16

B, S, D = 2, 2048, 1024
NH, DH = 16, 64
NCORES = 8
GROUPS = 4                # head-groups (cores per batch)
HG = NH // GROUPS         # heads per core = 4
IS = HG * DH              # inner slice per core = 256
KD = D // 128             # contraction chunks for projections = 8
MT = IS // 128            # head-pairs per core = 2
KT = S // 128             # 128-row key chunks = 16
WQ = 512                  # q-slab width
NSLAB = S // WQ           # q slabs = 4
NWARM = 16                # garbage warmup matmuls

_CACHE = {}


def _build_nc():
    nc = bacc.Bacc("TRN2", target_bir_lowering=False, debug=False)

    xqT = nc.dram_tensor("xqT", [D, S], F16, kind="ExternalInput").ap()
    xkT = nc.dram_tensor("xkT", [D, S], F16, kind="ExternalInput").ap()
    xvT = nc.dram_tensor("xvT", [D, S], F16, kind="ExternalInput").ap()
    wq = nc.dram_tensor("wq", [D, IS], F16, kind="ExternalInput").ap()
    wk = nc.dram_tensor("wk", [D, IS], F16, kind="ExternalInput").ap()
    wv = nc.dram_tensor("wv", [D, IS], F16, kind="ExternalInput").ap()
    wo = nc.dram_tensor("wo", [IS, D], F16, kind="ExternalInput").ap()
    bq = nc.dram_tensor("bq", [IS], F32, kind="ExternalInput").ap()
    bk = nc.dram_tensor("bk", [IS], F32, kind="ExternalInput").ap()
    out = nc.dram_tensor("out", [S, D], BF16, kind="ExternalOutput").ap()

    import os
    dbg = None
    if os.environ.get("MHA_DEBUG"):
        dbg = {
            "dbg_qt": nc.dram_tensor("dbg_qt", [128, MT, S], F16, kind="ExternalOutput").ap(),
            "dbg_kt": nc.dram_tensor("dbg_kt", [128, MT, S], F16, kind="ExternalOutput").ap(),
            "dbg_v": nc.dram_tensor("dbg_v", [128, KT, HG, 128], F16, kind="ExternalOutput").ap(),
            "dbg_ot": nc.dram_tensor("dbg_ot", [128, MT, S], F16, kind="ExternalOutput").ap(),
        }

    with tile.TileContext(nc) as tc:
        _emit(nc, tc, xqT, xkT, xvT, wq, wk, wv, wo, bq, bk, out, dbg)
    nc.compile()
    return nc


def _emit(nc, tc, xqT, xkT, xvT, wq, wk, wv, wo, bq, bk, out, dbg=None):
    from contextlib import ExitStack

    ctx = ExitStack()
    with ctx:
        consts = ctx.enter_context(tc.tile_pool(name="consts", bufs=1))
        big = ctx.enter_context(tc.tile_pool(name="big", bufs=1))
        xin = ctx.enter_context(tc.tile_pool(name="xin", bufs=4))
        mpool = ctx.enter_context(tc.tile_pool(name="mpool", bufs=3))
        expp = ctx.enter_context(tc.tile_pool(name="expp", bufs=4))
        smallp = ctx.enter_context(tc.tile_pool(name="smallp", bufs=4))
        bcsp = ctx.enter_context(tc.tile_pool(name="bcsp", bufs=2))
        outp = ctx.enter_context(tc.tile_pool(name="outp", bufs=3))

        # ---- constants ----
        col1_f = consts.tile([128, 1], F16, name="col1_f")
        nc.vector.memset(col1_f, 1.0)
        d1t = consts.tile([128, 1], F32, name="d1t")
        nc.vector.memset(d1t, D1)
        abias = consts.tile([128, 1], F32, name="abias")
        nc.vector.memset(abias, ACT_BIAS)
        garb_w = consts.tile([128, 128], F16, name="garb_w")
        garb_x = consts.tile([128, 512], F16, name="garb_x")
        nc.vector.memset(garb_w, 0.0)
        nc.vector.memset(garb_x, 0.0)
        dum = consts.tile([1, 2], F32, name="dum")
        nc.vector.memset(dum, 0.0)
        dum16 = consts.tile([1, 2], F16, name="dum16")
        wq_sb = consts.tile([128, KD, IS], F16, name="wq_sb")
        wk_sb = consts.tile([128, KD, IS], F16, name="wk_sb")
        wv_sb = consts.tile([128, KD, IS], F16, name="wv_sb")
        wo_sb = consts.tile([128, MT, D], F16, name="wo_sb")
        bq_sb = consts.tile([128, MT], F32, name="bq_sb")
        bk_sb = consts.tile([128, MT], F32, name="bk_sb")

        # ---- persistent intermediates ----
        # QT/KT pair-packed: partitions 0:64 = head 2m, 64:128 = head 2m+1
        QT_sb = big.tile([128, MT, S], F16, name="QT_sb")
        KT_sb = big.tile([128, MT, S], F16, name="KT_sb")
        # V natural per (chunk, head): [V(64)|1|0...] even, [1|0...|V(64)] odd
        V_sb = big.tile([128, KT, HG, 128], F16, name="V_sb")
        OT_sb = big.tile([128, MT, S], F16, name="OT_sb")
        nc.gpsimd.memset(V_sb, 0.0)
        for h in range(HG):
            c = DH if h % 2 == 0 else 0
            nc.vector.tensor_copy(
                V_sb[:, :, h, c:c + 1],
                col1_f.unsqueeze(1).broadcast_to([128, KT, 1]),
            )

        # trigger the ACT exp table DMA (~2.7us) during the initial input
        # DMAs instead of at the first real exp
        nc.scalar.activation(dum16, dum, mybir.ActivationFunctionType.Exp)

        # PSUM->SBUF moves alternating between DVE and ACT
        _eng = [0]

        def bias_copy(dst_ap, src_ap, bias_ap):
            if _eng[0] % 2 == 0:
                nc.vector.tensor_scalar_add(dst_ap, src_ap, bias_ap)
            else:
                nc.scalar.activation(
                    dst_ap, src_ap,
                    mybir.ActivationFunctionType.Identity,
                    bias=bias_ap,
                )
            _eng[0] += 1

        def plain_copy(dst_ap, src_ap):
            if _eng[0] % 2 == 0:
                nc.vector.tensor_copy(dst_ap, src_ap)
            else:
                nc.scalar.activation(
                    dst_ap, src_ap, mybir.ActivationFunctionType.Copy
                )
            _eng[0] += 1

        # ---- stage 1a: Q/K projections (transposed layout, pair-packed) ----
        NS2 = S // 512
        with nc.named_scope("proj"):
            with tc.tile_pool(name="psQK", bufs=8, space="PSUM") as psQK:
                # HAM warmup: garbage matmuls with no DMA dependencies run
                # immediately; ~3.4us of cold PE activity opens the clock
                # gate before the first real matmul needs it.
                for i in range(NWARM):
                    wt = psQK.tile([128, 512], F32, tag="ps", name=f"warm{i}")
                    nc.tensor.matmul(wt, garb_w, garb_x, start=True, stop=True)
                first_k = True
                for xT, w_dram, w_sb, b_dram, b_sb, dest in (
                    (xkT, wk, wk_sb, bk, bk_sb, KT_sb),
                    (xqT, wq, wq_sb, bq, bq_sb, QT_sb),
                ):
                    nc.sync.dma_start(
                        out=w_sb, in_=w_dram.rearrange("(k p) i -> p k i", p=128)
                    )
                    nc.sync.dma_start(
                        out=b_sb, in_=b_dram.rearrange("(m p) -> p m", p=128)
                    )
                    ps = [
                        [
                            psQK.tile([128, 512], F32, tag="ps", name=f"ps_{m}_{n}")
                            for n in range(NS2)
                        ]
                        for m in range(MT)
                    ]
                    for k in range(KD):
                        xt = xin.tile([128, S], F16, tag="xt")
                        nc.sync.dma_start(out=xt, in_=xT[128 * k:128 * (k + 1), :])
                        for m in range(MT):
                            for n in range(NS2):
                                nc.tensor.matmul(
                                    ps[m][n],
                                    w_sb[:, k, 128 * m:128 * (m + 1)],
                                    xt[:, 512 * n:512 * (n + 1)],
                                    start=(k == 0),
                                    stop=(k == KD - 1),
                                )
                    for m in range(MT):
                        for n in range(NS2):
                            bias_copy(
                                dest[:, m, 512 * n:512 * (n + 1)],
                                ps[m][n],
                                b_sb[:, m:m + 1],
                            )
                nc.sync.dma_start(
                    out=wv_sb, in_=wv.rearrange("(k p) i -> p k i", p=128)
                )

            # ---- stage 1b: V projection straight into natural layout ----
            # stationary = x^T seq-chunk, moving = Wv chunk; all 16 seq
            # tiles accumulate in PSUM at once (16 x 1KB = 8 banks).
            # matmul PSUM outputs must be bank-aligned (2KB); a [128, 256]
            # f32 tile takes a whole bank, so run two waves of 8 seq-tiles
            # with all 8 x^T chunks resident.
            with tc.tile_pool(name="psV", bufs=8, space="PSUM") as psVp, \
                 tc.tile_pool(name="xvp", bufs=KD) as xvp:
                xv_tiles = []
                for k in range(KD):
                    xt = xvp.tile([128, S], F16, tag="xv", name=f"xv{k}")
                    nc.sync.dma_start(out=xt, in_=xvT[128 * k:128 * (k + 1), :])
                    xv_tiles.append(xt)
                nc.sync.dma_start(
                    out=wo_sb, in_=wo.rearrange("(c p) d -> p c d", p=128)
                )
                for wave in range(2):
                    psV = [
                        psVp.tile([128, IS], F32, tag="v", name=f"psv{wave}_{jj}")
                        for jj in range(KT // 2)
                    ]
                    for k in range(KD):
                        for jj in range(KT // 2):
                            j = wave * (KT // 2) + jj
                            nc.tensor.matmul(
                                psV[jj],
                                xv_tiles[k][:, 128 * j:128 * (j + 1)],
                                wv_sb[:, k, :],
                                start=(k == 0),
                                stop=(k == KD - 1),
                            )
                    for jj in range(KT // 2):
                        j = wave * (KT // 2) + jj
                        for h in range(HG):
                            c = 0 if h % 2 == 0 else DH
                            plain_copy(
                                V_sb[:, j, h, c:c + DH],
                                psV[jj][:, DH * h:DH * (h + 1)],
                            )

        # ---- stage 2: attention, software-pipelined over (slab, pair, j).
        # Per iter: 2 concurrent score MMs -> one exp job (ACT or DVE
        # custom-op pair, alternating) -> 2 attnV MMs two iters later. ----
        with nc.named_scope("attn"):
            with tc.tile_pool(name="psAT", bufs=2, space="PSUM") as psAT:
                NIT = NSLAB * MT * KT
                LAG = 2
                exs = {}
                avs = {}
                _exp_eng = [0]

                def norm_half(av, m, s, hh):
                    # denominator row: 64 for even head, 0 for odd head
                    p0 = 0 if hh == 0 else DH
                    srow = DH if hh == 0 else 0
                    sums = smallp.tile([1, WQ], F32, tag="sums")
                    nc.vector.tensor_copy(sums, av[srow:srow + 1, hh, :])
                    rec = smallp.tile([1, WQ], F32, tag="rec")
                    nc.vector.reciprocal_approx_fast(rec, sums)
                    bcs = bcsp.tile([DH, WQ], F32, tag="bcs")
                    nc.gpsimd.partition_broadcast(bcs, rec)
                    nc.vector.tensor_mul(
                        OT_sb[p0:p0 + DH, m, WQ * s:WQ * (s + 1)],
                        av[p0:p0 + DH, hh, :],
                        bcs,
                    )

                for idx in range(NIT + LAG):
                    if idx < NIT:
                        s, mj = divmod(idx, MT * KT)
                        m, j = divmod(mj, KT)
                        sc = psAT.tile([128, 2, 512], F32, tag="sc", name="sc")
                        for hh in range(2):
                            p0 = 64 * hh
                            nc.tensor.matmul(
                                sc[:, hh, :],
                                KT_sb[p0:p0 + DH, m, 128 * j:128 * (j + 1)],
                                QT_sb[p0:p0 + DH, m, WQ * s:WQ * (s + 1)],
                                start=True, stop=True,
                            )
                        ex = expp.tile([128, 2, 512], F16, tag="ex")
                        if _exp_eng[0] % 2 == 0:
                            nc.scalar.activation(
                                ex, sc, mybir.ActivationFunctionType.Exp,
                                scale=0.125, bias=abias,
                            )
                        else:
                            mt = mpool.tile([128, 2, 512], F16, tag="m")
                            nc.vector._custom_dve(
                                EXP_P1, out=mt, in0=sc, in1=d1t,
                                s0=D4, s1=D3, imm2=D2,
                            )
                            nc.vector._custom_dve(EXP_SQSQ, out=ex, in0=mt)
                        _exp_eng[0] += 1
                        exs[(s, m, j)] = ex
                    if idx >= LAG:
                        s, mj = divmod(idx - LAG, MT * KT)
                        m, j = divmod(mj, KT)
                        if j == 0:
                            avs[(s, m)] = psAT.tile(
                                [128, 2, 512], F32, tag="av", name="av"
                            )
                        av = avs[(s, m)]
                        ex = exs.pop((s, m, j))
                        for hh in range(2):
                            nc.tensor.matmul(
                                av[:, hh, :],
                                V_sb[:, j, 2 * m + hh, :],
                                ex[:, hh, :],
                                start=(j == 0),
                                stop=(j == KT - 1),
                            )
                        if j == KT - 1:
                            av = avs.pop((s, m))
                            norm_half(av, m, s, 0)
                            norm_half(av, m, s, 1)

        if dbg is not None:
            nc.sync.dma_start(out=dbg["dbg_qt"], in_=QT_sb)
            nc.sync.dma_start(out=dbg["dbg_kt"], in_=KT_sb)
            nc.sync.dma_start(out=dbg["dbg_v"], in_=V_sb)
            nc.sync.dma_start(out=dbg["dbg_ot"], in_=OT_sb)

        # ---- stage 3: output projection tail; bf16 partials ----
        with nc.named_scope("outproj"):
            with tc.tile_pool(name="psFO", bufs=8, space="PSUM") as psFO:
                obs = {}
                for t in range(S // 128):
                    obs[t] = outp.tile([128, D], BF16, tag="ob", name="ob")
                    for half in range(2):
                        fo = psFO.tile([128, 512], F32, tag="fo", name="fo")
                        for m in range(MT):
                            nc.tensor.matmul(
                                fo,
                                OT_sb[:, m, 128 * t:128 * (t + 1)],
                                wo_sb[:, m, 512 * half:512 * (half + 1)],
                                start=(m == 0),
                                stop=(m == MT - 1),
                            )
                        plain_copy(obs[t][:, 512 * half:512 * (half + 1)], fo)
                    nc.sync.dma_start(
                        out=out[128 * t:128 * (t + 1), :], in_=obs.pop(t)
                    )


def _get_nc():
    if "nc" not in _CACHE:
        _CACHE["nc"] = _build_nc()
    return _CACHE["nc"]


def make_in_maps(query, key, value, Wq, bq, Wk, bk, Wv, bv, Wo, bo):
    f32 = lambda a: np.asarray(a, dtype=np.float32)
    f16 = lambda a: np.ascontiguousarray(np.asarray(a, dtype=np.float32).astype(NPDT))
    query, key, value = f32(query), f32(key), f32(value)
    bq, bk = np.ascontiguousarray(f32(bq)), np.ascontiguousarray(f32(bk))
    Wq, Wk, Wv, Wo = f32(Wq), f32(Wk), f32(Wv), f32(Wo)

    xT = [[f16(x[b].T) for b in range(B)] for x in (query, key, value)]
    in_maps = []
    for c in range(NCORES):
        b, g = c // GROUPS, c % GROUPS
        sl = slice(IS * g, IS * (g + 1))
        in_maps.append({
            "xqT": xT[0][b],
            "xkT": xT[1][b],
            "xvT": xT[2][b],
            "wq": f16(Wq[:, sl]),
            "wk": f16(Wk[:, sl]),
            "wv": f16(Wv[:, sl]),
            "wo": f16(Wo[sl, :]),
            "bq": np.ascontiguousarray(bq[sl]),
            "bk": np.ascontiguousarray(bk[sl]),
        })
    return in_maps


def combine_outputs(results, bv, Wo, bo):
    bo = np.asarray(bo, dtype=np.float32)
    bv = np.asarray(bv, dtype=np.float32)
    Wo = np.asarray(Wo, dtype=np.float32)
    bias = bo + bv @ Wo
    out = np.empty((B, S, D), dtype=np.float32)
    for b in range(B):
        acc = results[b * GROUPS]["out"].astype(np.float32)
        for g in range(1, GROUPS):
            acc = acc + results[b * GROUPS + g]["out"].astype(np.float32)
        out[b] = acc + bias
    return out


def kernel(query, key, value, Wq, bq, Wk, bk, Wv, bv, Wo, bo):
    nc = _get_nc()
    in_maps = make_in_maps(query, key, value, Wq, bq, Wk, bk, Wv, bv, Wo, bo)
    try:
        res = run_bass_kernel_spmd(nc, in_maps, list(range(NCORES)))
    except Exception:
        # a fresh NEFF's first execution occasionally reports
        # NRT_EXEC_UNIT_UNRECOVERABLE; a retry reliably succeeds
        res = run_bass_kernel_spmd(nc, in_maps, list(range(NCORES)))
    return combine_outputs(res.results, bv, Wo, bo)


# revision 13
# speedup vs baseline: 1.2441x; 1.2441x over previous
"""Multi-head attention (B=2, S=2048, D=1024, 16 heads x 64) on 8 NeuronCores.

Sharding: batch x head-group data/tensor parallel. Core c handles batch
c//4 and heads [4*(c%4), 4*(c%4)+4). Wq/Wk/Wv are column-sliced per head
group, Wo row-sliced; each core emits a partial [S, D] output (bf16) and
the host sums the 4 partials per batch and adds bo + bv@Wo (the V bias
commutes through attention since softmax rows sum to 1, so it is folded
into the final bias host-side).

v2 over the first working kernel (243.6us):
  1. Pair-packed Q^T/K^T: heads 2m / 2m+1 live in partitions 0:64 /
     64:128 of one [128, S] tile. The two heads' score matmuls (K=64
     contraction each) then run CONCURRENTLY in disjoint 32-row groups
     of the PE array (tile_position auto-derived from base_partition),
     halving scores PE time.
  2. exp split across ACT and a custom DVE op pair: P1 = deg-4 Horner
     q ~= exp(s/32)/c0 (8 ALU stages; coeffs via s0/s1/imm2 + C3 spill;
     constant c0 normalized away - softmax is scale-invariant, and the
     ACT path matches the scale via exp bias = -4*ln(c0)); P2 = q^4 via
     two squarings. ~0.01% poly error. ACT alone was the 136us phase
     bottleneck; split, both engines stay under the PE's attention time.
  3. V projected directly into natural [seq, dh] layout (stationary =
     x^T seq-chunk, moving = Wv), killing the PE transposes. Per-head
     128-col stationary blocks hold [V|1|0] (even head) / [1|0|V] (odd
     head) so attnV emits O^T rows partition-aligned for both heads and
     row 64/0 of the accumulator is the softmax denominator.
  4. Garbage-operand warmup matmuls at t=0 (no DMA deps) open the HAM
     clock gate (~3.4us busy window) before the real projections start;
     the whole PE stream is then gap-free so the gate never re-closes.
  5. Output projection as a short tail phase (PSUM too tight to drip it
     through attention); partials DMA'd as bf16 to halve the tail DMA.
"""

import sys

if "/opt/trn_rl_repo" not in sys.path:
    sys.path.insert(0, "/opt/trn_rl_repo")

import numpy as np

import concourse.bacc as bacc
import concourse.mybir as mybir
import concourse.tile as tile
from concourse.bass_utils import run_bass_kernel_spmd

# ---- custom DVE exp ops (registered at import; appended to the op table
# so existing rows keep their opcodes) --------------------------------------
import concourse.dve_ops as dve_ops
from concourse.dve_ops import DveOp
from concourse.dve_spec import (
    C0, C1, C2, One, Src0, Spec, sq, _has_src1, lower,
)
from concourse.dve_uop import DveOpSpec
from concourse.dve_table_gen import dve_ver_for

# cubic weighted-minimax fit of exp(s/32), normalized so the constant
# term is 1 (true constant c0 below; softmax cancels it, ACT compensates).
# One DVE instruction computes exp(s/8)/c0^4: Horner cubic (6 stages) +
# two squarings (2 stages) = the full 8-stage v3 budget.
D3, D2, D1 = (5.360408300406049e-06, 0.0005106809430822295,
              0.03125779746235493)
C0_NORM = 0.9988460037558242
ACT_BIAS = float(-4.0 * np.log(C0_NORM))  # exp(s/8 + bias) = exp(s/8)/c0^4


def _expc_ref(in0, in1, s0, s1, imm2):
    x = in0.astype(np.float32)
    q = ((s0 * x + s1) * x + imm2) * x + 1.0
    return (q * q) * (q * q)


def _register(name, body, ref):
    for op in dve_ops.OPS:
        if op.name == name:
            return op
    spec = Spec(body=body, reference=ref)
    row = dve_ops._CUSTOM_DVE_ROW_BASE + len(dve_ops.OPS)
    ver = dve_ver_for("TRN2")
    sha = DveOpSpec(
        name=name, opcode=row, uops=lower(spec, ver=ver),
        rd1_en=_has_src1(spec),
    ).sha(ver)
    op = DveOp(name, spec, subdim=False, uops_sha={ver: sha})
    dve_ops.OPS.append(op)
    dve_ops._SUB_OPCODE_FOR_NAME[name] = row
    dve_ops.CUSTOM_DVE_SPECS[name] = spec
    return op


EXP_C34 = _register(
    "EXP_C34_ANT",
    sq(sq((((C0 * Src0 + C1) * Src0 + C2) * Src0) + One)),
    _expc_ref,
)

F32 = mybir.dt.float32
F16 = mybir.dt.float16
BF16 = mybir.dt.bfloat16
NPDT = np.float16

B, S, D = 2, 2048, 1024
NH, DH = 16, 64
NCORES = 8
GROUPS = 4                # head-groups (cores per batch)
HG = NH // GROUPS         # heads per core = 4
IS = HG * DH              # inner slice per core = 256
KD = D // 128             # contraction chunks for projections = 8
MT = IS // 128            # head-pairs per core = 2
KT = S // 128             # 128-row key chunks = 16
WQ = 512                  # q-slab width
NSLAB = S // WQ           # q slabs = 4
NWARM = 16                # garbage warmup matmuls
ACT_JOBS = 74             # exp jobs on ACT (rest on the DVE custom op)

_CACHE = {}


def _build_nc():
    nc = bacc.Bacc("TRN2", target_bir_lowering=False, debug=False)

    xqT = nc.dram_tensor("xqT", [D, S], F16, kind="ExternalInput").ap()
    xkT = nc.dram_tensor("xkT", [D, S], F16, kind="ExternalInput").ap()
    xvT = nc.dram_tensor("xvT", [D, S], F16, kind="ExternalInput").ap()
    wq = nc.dram_tensor("wq", [D, IS], F16, kind="ExternalInput").ap()
    wk = nc.dram_tensor("wk", [D, IS], F16, kind="ExternalInput").ap()
    wv = nc.dram_tensor("wv", [D, IS], F16, kind="ExternalInput").ap()
    wo = nc.dram_tensor("wo", [IS, D], F16, kind="ExternalInput").ap()
    bq = nc.dram_tensor("bq", [IS], F32, kind="ExternalInput").ap()
    bk = nc.dram_tensor("bk", [IS], F32, kind="ExternalInput").ap()
    out = nc.dram_tensor("out", [S, D], BF16, kind="ExternalOutput").ap()

    import os
    dbg = None
    if os.environ.get("MHA_DEBUG"):
        dbg = {
            "dbg_qt": nc.dram_tensor("dbg_qt", [128, MT, S], F16, kind="ExternalOutput").ap(),
            "dbg_kt": nc.dram_tensor("dbg_kt", [128, MT, S], F16, kind="ExternalOutput").ap(),
            "dbg_v": nc.dram_tensor("dbg_v", [128, KT, HG, 128], F16, kind="ExternalOutput").ap(),
            "dbg_ot": nc.dram_tensor("dbg_ot", [128, MT, S], F16, kind="ExternalOutput").ap(),
        }

    with tile.TileContext(nc) as tc:
        _emit(nc, tc, xqT, xkT, xvT, wq, wk, wv, wo, bq, bk, out, dbg)
    nc.compile()
    return nc


def _emit(nc, tc, xqT, xkT, xvT, wq, wk, wv, wo, bq, bk, out, dbg=None):
    from contextlib import ExitStack

    ctx = ExitStack()
    with ctx:
        consts = ctx.enter_context(tc.tile_pool(name="consts", bufs=1))
        big = ctx.enter_context(tc.tile_pool(name="big", bufs=1))
        xin = ctx.enter_context(tc.tile_pool(name="xin", bufs=4))
        expp = ctx.enter_context(tc.tile_pool(name="expp", bufs=4))
        smallp = ctx.enter_context(tc.tile_pool(name="smallp", bufs=4))
        bcsp = ctx.enter_context(tc.tile_pool(name="bcsp", bufs=2))
        outp = ctx.enter_context(tc.tile_pool(name="outp", bufs=3))

        # ---- constants ----
        col1_f = consts.tile([128, 1], F16, name="col1_f")
        nc.vector.memset(col1_f, 1.0)
        abias = consts.tile([128, 1], F32, name="abias")
        nc.vector.memset(abias, ACT_BIAS)
        garb_w = consts.tile([128, 128], F16, name="garb_w")
        garb_x = consts.tile([128, 512], F16, name="garb_x")
        nc.vector.memset(garb_w, 0.0)
        nc.vector.memset(garb_x, 0.0)
        dum = consts.tile([1, 2], F32, name="dum")
        nc.vector.memset(dum, 0.0)
        dum16 = consts.tile([1, 2], F16, name="dum16")
        wq_sb = consts.tile([128, KD, IS], F16, name="wq_sb")
        wk_sb = consts.tile([128, KD, IS], F16, name="wk_sb")
        wv_sb = consts.tile([128, KD, IS], F16, name="wv_sb")
        wo_sb = consts.tile([128, MT, D], F16, name="wo_sb")
        bq_sb = consts.tile([128, MT], F32, name="bq_sb")
        bk_sb = consts.tile([128, MT], F32, name="bk_sb")

        # ---- persistent intermediates ----
        # QT/KT pair-packed: partitions 0:64 = head 2m, 64:128 = head 2m+1
        QT_sb = big.tile([128, MT, S], F16, name="QT_sb")
        KT_sb = big.tile([128, MT, S], F16, name="KT_sb")
        VT_sb = big.tile([128, MT, S], F16, name="VT_sb")
        # V natural per (chunk, head): [V(64)|1|0...] even, [1|0...|V(64)] odd
        V_sb = big.tile([128, KT, HG, 128], F16, name="V_sb")
        OT_sb = big.tile([128, MT, S], F16, name="OT_sb")
        nc.gpsimd.memset(V_sb, 0.0)
        for h in range(HG):
            c = DH if h % 2 == 0 else 0
            nc.vector.tensor_copy(
                V_sb[:, :, h, c:c + 1],
                col1_f.unsqueeze(1).broadcast_to([128, KT, 1]),
            )

        # trigger the ACT exp table DMA (~2.7us) during the initial input
        # DMAs instead of at the first real exp
        nc.scalar.activation(dum16, dum, mybir.ActivationFunctionType.Exp)

        # PSUM->SBUF moves alternating between DVE and ACT
        _eng = [0]

        def bias_copy(dst_ap, src_ap, bias_ap):
            if _eng[0] % 2 == 0:
                nc.vector.tensor_scalar_add(dst_ap, src_ap, bias_ap)
            else:
                nc.scalar.activation(
                    dst_ap, src_ap,
                    mybir.ActivationFunctionType.Identity,
                    bias=bias_ap,
                )
            _eng[0] += 1

        def plain_copy(dst_ap, src_ap):
            if _eng[0] % 2 == 0:
                nc.vector.tensor_copy(dst_ap, src_ap)
            else:
                nc.scalar.activation(
                    dst_ap, src_ap, mybir.ActivationFunctionType.Copy
                )
            _eng[0] += 1

        # ---- stage 1a: Q/K projections (transposed layout, pair-packed) ----
        NS2 = S // 512
        with nc.named_scope("proj"):
            with tc.tile_pool(name="psQK", bufs=8, space="PSUM") as psQK:
                # HAM warmup: garbage matmuls with no DMA dependencies run
                # immediately; ~3.4us of cold PE activity opens the clock
                # gate before the first real matmul needs it.
                for i in range(NWARM):
                    wt = psQK.tile([128, 512], F32, tag="ps", name=f"warm{i}")
                    nc.tensor.matmul(wt, garb_w, garb_x, start=True, stop=True)
                for xT, w_dram, w_sb, b_dram, b_sb, dest in (
                    (xkT, wk, wk_sb, bk, bk_sb, KT_sb),
                    (xqT, wq, wq_sb, bq, bq_sb, QT_sb),
                    (xvT, wv, wv_sb, None, None, VT_sb),
                ):
                    nc.sync.dma_start(
                        out=w_sb, in_=w_dram.rearrange("(k p) i -> p k i", p=128)
                    )
                    if b_dram is not None:
                        nc.sync.dma_start(
                            out=b_sb, in_=b_dram.rearrange("(m p) -> p m", p=128)
                        )
                    ps = [
                        [
                            psQK.tile([128, 512], F32, tag="ps", name=f"ps_{m}_{n}")
                            for n in range(NS2)
                        ]
                        for m in range(MT)
                    ]
                    for k in range(KD):
                        xt = xin.tile([128, S], F16, tag="xt")
                        nc.sync.dma_start(out=xt, in_=xT[128 * k:128 * (k + 1), :])
                        for m in range(MT):
                            for n in range(NS2):
                                nc.tensor.matmul(
                                    ps[m][n],
                                    w_sb[:, k, 128 * m:128 * (m + 1)],
                                    xt[:, 512 * n:512 * (n + 1)],
                                    start=(k == 0),
                                    stop=(k == KD - 1),
                                )
                    for m in range(MT):
                        for n in range(NS2):
                            if b_sb is None:
                                plain_copy(
                                    dest[:, m, 512 * n:512 * (n + 1)], ps[m][n]
                                )
                            else:
                                bias_copy(
                                    dest[:, m, 512 * n:512 * (n + 1)],
                                    ps[m][n],
                                    b_sb[:, m:m + 1],
                                )
                nc.sync.dma_start(
                    out=wo_sb, in_=wo.rearrange("(c p) d -> p c d", p=128)
                )
            # V^T -> V natural via DMA-transpose XBAR (off the PE and off
            # ACT/DVE): one [64, S] -> [S(=128x16), 64] transpose per head,
            # written straight into the strided per-head V blocks.
            for h in range(HG):
                c = 0 if h % 2 == 0 else DH
                nc.sync.dma_start(
                    out=V_sb[:, :, h, c:c + DH],
                    in_=VT_sb[64 * (h % 2):64 * (h % 2) + DH, h // 2, :],
                    transpose=True,
                )

            # ---- stage 1b: V projection straight into natural layout ----
            # stationary = x^T seq-chunk, moving = Wv chunk; all 16 seq
            # tiles accumulate in PSUM at once (16 x 1KB = 8 banks).
        # ---- stage 2: attention, software-pipelined over (slab, pair, j).
        # Per iter: 2 concurrent score MMs -> one exp job (ACT or DVE
        # custom-op pair, alternating) -> 2 attnV MMs two iters later. ----
        with nc.named_scope("attn"):
            with tc.tile_pool(name="psAT", bufs=2, space="PSUM") as psAT:
                NIT = NSLAB * MT * KT
                LAG = 2
                exs = {}
                avs = {}
                _exp_eng = [0]

                def norm_group(av, m, s):
                    # denominator rows: 64 (even head) / 0 (odd head)
                    for hh in range(2):
                        p0 = 0 if hh == 0 else DH
                        srow = DH if hh == 0 else 0
                        sums = smallp.tile([1, WQ], F32, tag="sums")
                        nc.scalar.activation(
                            sums, av[srow:srow + 1, hh, :],
                            mybir.ActivationFunctionType.Copy,
                        )
                        rec = smallp.tile([1, WQ], F32, tag="rec")
                        nc.vector.reciprocal_approx_fast(rec, sums)
                        bcs = bcsp.tile([DH, WQ], F32, tag="bcs")
                        nc.gpsimd.partition_broadcast(bcs, rec)
                        nc.vector.tensor_mul(
                            OT_sb[p0:p0 + DH, m, WQ * s:WQ * (s + 1)],
                            av[p0:p0 + DH, hh, :],
                            bcs,
                        )

                for idx in range(NIT + LAG):
                    if idx < NIT:
                        s, mj = divmod(idx, MT * KT)
                        m, j = divmod(mj, KT)
                        sc = psAT.tile([128, 2, 512], F32, tag="sc", name="sc")
                        for hh in range(2):
                            p0 = 64 * hh
                            nc.tensor.matmul(
                                sc[:, hh, :],
                                KT_sb[p0:p0 + DH, m, 128 * j:128 * (j + 1)],
                                QT_sb[p0:p0 + DH, m, WQ * s:WQ * (s + 1)],
                                start=True, stop=True,
                            )
                        ex = expp.tile([128, 2, 512], F16, tag="ex")
                        i_e = _exp_eng[0]
                        on_act = (i_e * ACT_JOBS) // NIT != ((i_e + 1) * ACT_JOBS) // NIT
                        if on_act:
                            nc.scalar.activation(
                                ex, sc, mybir.ActivationFunctionType.Exp,
                                scale=0.125, bias=abias,
                            )
                        else:
                            nc.vector._custom_dve(
                                EXP_C34, out=ex, in0=sc,
                                s0=D3, s1=D2, imm2=D1,
                            )
                        _exp_eng[0] += 1
                        exs[(s, m, j)] = ex
                    if idx >= LAG:
                        s, mj = divmod(idx - LAG, MT * KT)
                        m, j = divmod(mj, KT)
                        if j == 0:
                            avs[(s, m)] = psAT.tile(
                                [128, 2, 512], F32, tag="av", name="av"
                            )
                        av = avs[(s, m)]
                        ex = exs.pop((s, m, j))
                        for hh in range(2):
                            nc.tensor.matmul(
                                av[:, hh, :],
                                V_sb[:, j, 2 * m + hh, :],
                                ex[:, hh, :],
                                start=(j == 0),
                                stop=(j == KT - 1),
                            )
                        if j == KT - 1:
                            av = avs.pop((s, m))
                            norm_group(av, m, s)

        if dbg is not None:
            nc.sync.dma_start(out=dbg["dbg_qt"], in_=QT_sb)
            nc.sync.dma_start(out=dbg["dbg_kt"], in_=KT_sb)
            nc.sync.dma_start(out=dbg["dbg_v"], in_=V_sb)
            nc.sync.dma_start(out=dbg["dbg_ot"], in_=OT_sb)

        # ---- stage 3: output projection tail; bf16 partials ----
        with nc.named_scope("outproj"):
            with tc.tile_pool(name="psFO", bufs=8, space="PSUM") as psFO:
                nfo = [0]
                for t in range(S // 128):
                    ob = outp.tile([128, D], BF16, tag="ob", name="ob")
                    for half in range(2):
                        fo = psFO.tile([128, 512], F32, tag="fo", name="fo")
                        for m in range(MT):
                            nc.tensor.matmul(
                                fo,
                                OT_sb[:, m, 128 * t:128 * (t + 1)],
                                wo_sb[:, m, 512 * half:512 * (half + 1)],
                                start=(m == 0),
                                stop=(m == MT - 1),
                            )
                        dst = ob[:, 512 * half:512 * (half + 1)]
                        # DVE's f32->bf16 CAST (~376ns) is ~2x faster than
                        # ACT's copy; weight the split 2:1 toward DVE
                        if nfo[0] % 3 == 2:
                            nc.scalar.activation(
                                dst, fo, mybir.ActivationFunctionType.Copy
                            )
                        else:
                            nc.vector.tensor_copy(dst, fo)
                        nfo[0] += 1
                        nc.sync.dma_start(
                            out=out[128 * t:128 * (t + 1),
                                    512 * half:512 * (half + 1)],
                            in_=dst,
                        )


def _get_nc():
    if "nc" not in _CACHE:
        _CACHE["nc"] = _build_nc()
    return _CACHE["nc"]


def make_in_maps(query, key, value, Wq, bq, Wk, bk, Wv, bv, Wo, bo):
    f32 = lambda a: np.asarray(a, dtype=np.float32)
    f16 = lambda a: np.ascontiguousarray(np.asarray(a, dtype=np.float32).astype(NPDT))
    query, key, value = f32(query), f32(key), f32(value)
    bq, bk = np.ascontiguousarray(f32(bq)), np.ascontiguousarray(f32(bk))
    Wq, Wk, Wv, Wo = f32(Wq), f32(Wk), f32(Wv), f32(Wo)

    xT = [[f16(x[b].T) for b in range(B)] for x in (query, key, value)]
    in_maps = []
    for c in range(NCORES):
        b, g = c // GROUPS, c % GROUPS
        sl = slice(IS * g, IS * (g + 1))
        in_maps.append({
            "xqT": xT[0][b],
            "xkT": xT[1][b],
            "xvT": xT[2][b],
            "wq": f16(Wq[:, sl]),
            "wk": f16(Wk[:, sl]),
            "wv": f16(Wv[:, sl]),
            "wo": f16(Wo[sl, :]),
            "bq": np.ascontiguousarray(bq[sl]),
            "bk": np.ascontiguousarray(bk[sl]),
        })
    return in_maps


def combine_outputs(results, bv, Wo, bo):
    bo = np.asarray(bo, dtype=np.float32)
    bv = np.asarray(bv, dtype=np.float32)
    Wo = np.asarray(Wo, dtype=np.float32)
    bias = bo + bv @ Wo
    out = np.empty((B, S, D), dtype=np.float32)
    for b in range(B):
        acc = results[b * GROUPS]["out"].astype(np.float32)
        for g in range(1, GROUPS):
            acc = acc + results[b * GROUPS + g]["out"].astype(np.float32)
        out[b] = acc + bias
    return out


def kernel(query, key, value, Wq, bq, Wk, bk, Wv, bv, Wo, bo):
    nc = _get_nc()
    in_maps = make_in_maps(query, key, value, Wq, bq, Wk, bk, Wv, bv, Wo, bo)
    try:
        res = run_bass_kernel_spmd(nc, in_maps, list(range(NCORES)))
    except Exception:
        # a fresh NEFF's first execution occasionally reports
        # NRT_EXEC_UNIT_UNRECOVERABLE; a retry reliably succeeds
        res = run_bass_kernel_spmd(nc, in_maps, list(range(NCORES)))
    return combine_outputs(res.results, bv, Wo, bo)


# revision 14
# speedup vs baseline: 1.3565x; 1.0904x over previous
"""Multi-head attention (B=2, S=2048, D=1024, 16 heads x 64) on 8 NeuronCores.

Sharding: batch x head-group data/tensor parallel. Core c handles batch
c//4 and heads [4*(c%4), 4*(c%4)+4). Wq/Wk/Wv are column-sliced per head
group, Wo row-sliced; each core emits a partial [S, D] output (bf16) and
the host sums the 4 partials per batch and adds bo + bv@Wo (the V bias
commutes through attention since softmax rows sum to 1, so it is folded
into the final bias host-side).

v2 over the first working kernel (243.6us):
  1. Pair-packed Q^T/K^T: heads 2m / 2m+1 live in partitions 0:64 /
     64:128 of one [128, S] tile. The two heads' score matmuls (K=64
     contraction each) then run CONCURRENTLY in disjoint 32-row groups
     of the PE array (tile_position auto-derived from base_partition),
     halving scores PE time.
  2. exp split across ACT and a custom DVE op pair: P1 = deg-4 Horner
     q ~= exp(s/32)/c0 (8 ALU stages; coeffs via s0/s1/imm2 + C3 spill;
     constant c0 normalized away - softmax is scale-invariant, and the
     ACT path matches the scale via exp bias = -4*ln(c0)); P2 = q^4 via
     two squarings. ~0.01% poly error. ACT alone was the 136us phase
     bottleneck; split, both engines stay under the PE's attention time.
  3. V projected directly into natural [seq, dh] layout (stationary =
     x^T seq-chunk, moving = Wv), killing the PE transposes. Per-head
     128-col stationary blocks hold [V|1|0] (even head) / [1|0|V] (odd
     head) so attnV emits O^T rows partition-aligned for both heads and
     row 64/0 of the accumulator is the softmax denominator.
  4. Garbage-operand warmup matmuls at t=0 (no DMA deps) open the HAM
     clock gate (~3.4us busy window) before the real projections start;
     the whole PE stream is then gap-free so the gate never re-closes.
  5. Output projection as a short tail phase (PSUM too tight to drip it
     through attention); partials DMA'd as bf16 to halve the tail DMA.
"""

import sys

if "/opt/trn_rl_repo" not in sys.path:
    sys.path.insert(0, "/opt/trn_rl_repo")

import numpy as np

import concourse.bacc as bacc
import concourse.mybir as mybir
import concourse.tile as tile
from concourse.bass_utils import run_bass_kernel_spmd

# ---- custom DVE exp ops (registered at import; appended to the op table
# so existing rows keep their opcodes) --------------------------------------
import concourse.dve_ops as dve_ops
from concourse.dve_ops import DveOp
from concourse.dve_spec import (
    C0, C1, C2, One, Src0, Spec, sq, _has_src1, lower,
)
from concourse.dve_uop import DveOpSpec
from concourse.dve_table_gen import dve_ver_for

# cubic weighted-minimax fit of exp(s/32), normalized so the constant
# term is 1 (true constant c0 below; softmax cancels it, ACT compensates).
# One DVE instruction computes exp(s/8)/c0^4: Horner cubic (6 stages) +
# two squarings (2 stages) = the full 8-stage v3 budget.
D3, D2, D1 = (5.360408300406049e-06, 0.0005106809430822295,
              0.03125779746235493)
C0_NORM = 0.9988460037558242
ACT_BIAS = float(-4.0 * np.log(C0_NORM))  # exp(s/8 + bias) = exp(s/8)/c0^4


def _expc_ref(in0, in1, s0, s1, imm2):
    x = in0.astype(np.float32)
    q = ((s0 * x + s1) * x + imm2) * x + 1.0
    return (q * q) * (q * q)


def _register(name, body, ref):
    for op in dve_ops.OPS:
        if op.name == name:
            return op
    spec = Spec(body=body, reference=ref)
    row = dve_ops._CUSTOM_DVE_ROW_BASE + len(dve_ops.OPS)
    ver = dve_ver_for("TRN2")
    sha = DveOpSpec(
        name=name, opcode=row, uops=lower(spec, ver=ver),
        rd1_en=_has_src1(spec),
    ).sha(ver)
    op = DveOp(name, spec, subdim=False, uops_sha={ver: sha})
    dve_ops.OPS.append(op)
    dve_ops._SUB_OPCODE_FOR_NAME[name] = row
    dve_ops.CUSTOM_DVE_SPECS[name] = spec
    return op


EXP_C34 = _register(
    "EXP_C34_ANT",
    sq(sq((((C0 * Src0 + C1) * Src0 + C2) * Src0) + One)),
    _expc_ref,
)

F32 = mybir.dt.float32
F16 = mybir.dt.float16
BF16 = mybir.dt.bfloat16
NPDT = np.float16

B, S, D = 2, 2048, 1024
NH, DH = 16, 64
NCORES = 8
GROUPS = 4                # head-groups (cores per batch)
HG = NH // GROUPS         # heads per core = 4
IS = HG * DH              # inner slice per core = 256
KD = D // 128             # contraction chunks for projections = 8
MT = IS // 128            # head-pairs per core = 2
KT = S // 128             # 128-row key chunks = 16
WQ = 512                  # q-slab width
NSLAB = S // WQ           # q slabs = 4
NWARM = 16                # garbage warmup matmuls
ACT_JOBS = 74             # exp jobs on ACT (rest on the DVE custom op)

_CACHE = {}


def _build_nc():
    nc = bacc.Bacc("TRN2", target_bir_lowering=False, debug=False)

    xqT = nc.dram_tensor("xqT", [D, S], F16, kind="ExternalInput").ap()
    xkT = nc.dram_tensor("xkT", [D, S], F16, kind="ExternalInput").ap()
    xvT = nc.dram_tensor("xvT", [D, S], F16, kind="ExternalInput").ap()
    wq = nc.dram_tensor("wq", [D, IS], F16, kind="ExternalInput").ap()
    wk = nc.dram_tensor("wk", [D, IS], F16, kind="ExternalInput").ap()
    wv = nc.dram_tensor("wv", [D, IS], F16, kind="ExternalInput").ap()
    wo = nc.dram_tensor("wo", [IS, D], F16, kind="ExternalInput").ap()
    bq = nc.dram_tensor("bq", [IS], F32, kind="ExternalInput").ap()
    bk = nc.dram_tensor("bk", [IS], F32, kind="ExternalInput").ap()
    out = nc.dram_tensor("out", [S, D], BF16, kind="ExternalOutput").ap()

    import os
    dbg = None
    if os.environ.get("MHA_DEBUG"):
        dbg = {
            "dbg_qt": nc.dram_tensor("dbg_qt", [128, MT, S], F16, kind="ExternalOutput").ap(),
            "dbg_kt": nc.dram_tensor("dbg_kt", [128, MT, S], F16, kind="ExternalOutput").ap(),
            "dbg_v": nc.dram_tensor("dbg_v", [128, KT, HG, 128], F16, kind="ExternalOutput").ap(),
            "dbg_ot": nc.dram_tensor("dbg_ot", [128, MT, S], F16, kind="ExternalOutput").ap(),
        }

    with tile.TileContext(nc) as tc:
        _emit(nc, tc, xqT, xkT, xvT, wq, wk, wv, wo, bq, bk, out, dbg)
    nc.compile()
    return nc


def _emit(nc, tc, xqT, xkT, xvT, wq, wk, wv, wo, bq, bk, out, dbg=None):
    from contextlib import ExitStack

    ctx = ExitStack()
    with ctx:
        consts = ctx.enter_context(tc.tile_pool(name="consts", bufs=1))
        big = ctx.enter_context(tc.tile_pool(name="big", bufs=1))
        xin = ctx.enter_context(tc.tile_pool(name="xin", bufs=4))
        expp = ctx.enter_context(tc.tile_pool(name="expp", bufs=6))
        smallp = ctx.enter_context(tc.tile_pool(name="smallp", bufs=4))
        bcsp = ctx.enter_context(tc.tile_pool(name="bcsp", bufs=2))
        outp = ctx.enter_context(tc.tile_pool(name="outp", bufs=3))

        # ---- constants ----
        col1_f = consts.tile([128, 1], F16, name="col1_f")
        nc.vector.memset(col1_f, 1.0)
        abias = consts.tile([128, 1], F32, name="abias")
        nc.vector.memset(abias, ACT_BIAS)
        garb_w = consts.tile([128, 128], F16, name="garb_w")
        garb_x = consts.tile([128, 512], F16, name="garb_x")
        nc.vector.memset(garb_w, 0.0)
        nc.vector.memset(garb_x, 0.0)
        dum = consts.tile([1, 2], F32, name="dum")
        nc.vector.memset(dum, 0.0)
        dum16 = consts.tile([1, 2], F16, name="dum16")
        wq_sb = consts.tile([128, KD, IS], F16, name="wq_sb")
        wk_sb = consts.tile([128, KD, IS], F16, name="wk_sb")
        wv_sb = consts.tile([128, KD, IS], F16, name="wv_sb")
        wo_sb = consts.tile([128, MT, D], F16, name="wo_sb")
        bq_sb = consts.tile([128, MT], F32, name="bq_sb")
        bk_sb = consts.tile([128, MT], F32, name="bk_sb")

        # ---- persistent intermediates ----
        # QT/KT pair-packed: partitions 0:64 = head 2m, 64:128 = head 2m+1
        QT_sb = [
            big.tile([128, MT, WQ], F16, name=f"QT_sb{s}") for s in range(NSLAB)
        ]
        KT_sb = big.tile([128, MT, S], F16, name="KT_sb")
        VT_sb = big.tile([128, MT, S], F16, name="VT_sb")
        # V natural per (chunk, head): [V(64)|1|0...] even, [1|0...|V(64)] odd
        V_sb = [
            big.tile([128, KT, 128], F16, name=f"V_sb{h}") for h in range(HG)
        ]
        OT_sb = [
            big.tile([128, MT, WQ], F16, name=f"OT_sb{s}") for s in range(NSLAB)
        ]
        for h in range(HG):
            nc.gpsimd.memset(V_sb[h], 0.0)
            c = DH if h % 2 == 0 else 0
            nc.vector.tensor_copy(
                V_sb[h][:, :, c:c + 1],
                col1_f.unsqueeze(1).broadcast_to([128, KT, 1]),
            )

        # trigger the ACT exp table DMA (~2.7us) during the initial input
        # DMAs instead of at the first real exp
        nc.scalar.activation(dum16, dum, mybir.ActivationFunctionType.Exp)

        # PSUM->SBUF moves alternating between DVE and ACT
        _eng = [0]

        def bias_copy(dst_ap, src_ap, bias_ap):
            if _eng[0] % 2 == 0:
                nc.vector.tensor_scalar_add(dst_ap, src_ap, bias_ap)
            else:
                nc.scalar.activation(
                    dst_ap, src_ap,
                    mybir.ActivationFunctionType.Identity,
                    bias=bias_ap,
                )
            _eng[0] += 1

        def plain_copy(dst_ap, src_ap):
            if _eng[0] % 2 == 0:
                nc.vector.tensor_copy(dst_ap, src_ap)
            else:
                nc.scalar.activation(
                    dst_ap, src_ap, mybir.ActivationFunctionType.Copy
                )
            _eng[0] += 1

        # ---- stage 1a: Q/K projections (transposed layout, pair-packed) ----
        NS2 = S // 512
        with nc.named_scope("proj"):
            with tc.tile_pool(name="psQK", bufs=8, space="PSUM") as psQK:
                # HAM warmup: garbage matmuls with no DMA dependencies run
                # immediately; ~3.4us of cold PE activity opens the clock
                # gate before the first real matmul needs it.
                for i in range(NWARM):
                    wt = psQK.tile([128, 512], F32, tag="ps", name=f"warm{i}")
                    nc.tensor.matmul(wt, garb_w, garb_x, start=True, stop=True)
                for xT, w_dram, w_sb, b_dram, b_sb, dest in (
                    (xkT, wk, wk_sb, bk, bk_sb, KT_sb),
                    (xqT, wq, wq_sb, bq, bq_sb, QT_sb),
                    (xvT, wv, wv_sb, None, None, VT_sb),
                ):
                    nc.sync.dma_start(
                        out=w_sb, in_=w_dram.rearrange("(k p) i -> p k i", p=128)
                    )
                    if b_dram is not None:
                        nc.sync.dma_start(
                            out=b_sb, in_=b_dram.rearrange("(m p) -> p m", p=128)
                        )
                    ps = [
                        [
                            psQK.tile([128, 512], F32, tag="ps", name=f"ps_{m}_{n}")
                            for n in range(NS2)
                        ]
                        for m in range(MT)
                    ]
                    for k in range(KD):
                        xt = xin.tile([128, S], F16, tag="xt")
                        nc.sync.dma_start(out=xt, in_=xT[128 * k:128 * (k + 1), :])
                        for m in range(MT):
                            for n in range(NS2):
                                nc.tensor.matmul(
                                    ps[m][n],
                                    w_sb[:, k, 128 * m:128 * (m + 1)],
                                    xt[:, 512 * n:512 * (n + 1)],
                                    start=(k == 0),
                                    stop=(k == KD - 1),
                                )
                    for m in range(MT):
                        for n in range(NS2):
                            if dest is QT_sb:
                                dst = QT_sb[n][:, m, :]
                            else:
                                dst = dest[:, m, 512 * n:512 * (n + 1)]
                            if b_sb is None:
                                plain_copy(dst, ps[m][n])
                            else:
                                bias_copy(dst, ps[m][n], b_sb[:, m:m + 1])
                nc.sync.dma_start(
                    out=wo_sb, in_=wo.rearrange("(c p) d -> p c d", p=128)
                )
            # V^T -> V natural via DMA-transpose XBAR (off the PE and off
            # ACT/DVE): one [64, S] -> [S(=128x16), 64] transpose per head,
            # written straight into the strided per-head V blocks.
            for h in range(HG):
                c = 0 if h % 2 == 0 else DH
                nc.sync.dma_start(
                    out=V_sb[h][:, :, c:c + DH],
                    in_=VT_sb[64 * (h % 2):64 * (h % 2) + DH, h // 2, :],
                    transpose=True,
                )

            # ---- stage 1b: V projection straight into natural layout ----
            # stationary = x^T seq-chunk, moving = Wv chunk; all 16 seq
            # tiles accumulate in PSUM at once (16 x 1KB = 8 banks).
        # ---- stage 2: attention, software-pipelined over (slab, pair, j).
        # Per iter: 2 concurrent score MMs -> one exp job (ACT or DVE
        # custom-op pair, alternating) -> 2 attnV MMs two iters later. ----
        with nc.named_scope("attn"):
            with tc.tile_pool(name="psAT", bufs=2, space="PSUM") as psAT:
                NIT = NSLAB * MT * KT
                LAG = 2
                exs = {}
                avs = {}
                _exp_eng = [0]

                def norm_group(av, m, s):
                    # denominator rows: 64 (even head) / 0 (odd head)
                    for hh in range(2):
                        p0 = 0 if hh == 0 else DH
                        srow = DH if hh == 0 else 0
                        sums = smallp.tile([1, WQ], F32, tag="sums")
                        nc.scalar.activation(
                            sums, av[srow:srow + 1, hh, :],
                            mybir.ActivationFunctionType.Copy,
                        )
                        rec = smallp.tile([1, WQ], F32, tag="rec")
                        nc.vector.reciprocal_approx_fast(rec, sums)
                        bcs = bcsp.tile([DH, WQ], F32, tag="bcs")
                        nc.gpsimd.partition_broadcast(bcs, rec)
                        nc.vector.tensor_mul(
                            OT_sb[s][p0:p0 + DH, m, :],
                            av[p0:p0 + DH, hh, :],
                            bcs,
                        )

                for idx in range(NIT + LAG):
                    if idx < NIT:
                        s, mj = divmod(idx, MT * KT)
                        m, j = divmod(mj, KT)
                        sc = psAT.tile([128, 2, 512], F32, tag="sc", name="sc")
                        for hh in range(2):
                            p0 = 64 * hh
                            nc.tensor.matmul(
                                sc[:, hh, :],
                                KT_sb[p0:p0 + DH, m, 128 * j:128 * (j + 1)],
                                QT_sb[s][p0:p0 + DH, m, :],
                                start=True, stop=True,
                            )
                        ex = expp.tile([128, 2, 512], F16, tag="ex")
                        i_e = _exp_eng[0]
                        on_act = (i_e * ACT_JOBS) // NIT != ((i_e + 1) * ACT_JOBS) // NIT
                        if on_act:
                            nc.scalar.activation(
                                ex, sc, mybir.ActivationFunctionType.Exp,
                                scale=0.125, bias=abias,
                            )
                        else:
                            nc.vector._custom_dve(
                                EXP_C34, out=ex, in0=sc,
                                s0=D3, s1=D2, imm2=D1,
                            )
                        _exp_eng[0] += 1
                        exs[(s, m, j)] = ex
                    if idx >= LAG:
                        s, mj = divmod(idx - LAG, MT * KT)
                        m, j = divmod(mj, KT)
                        if j == 0:
                            avs[(s, m)] = psAT.tile(
                                [128, 2, 512], F32, tag="av", name="av"
                            )
                        av = avs[(s, m)]
                        ex = exs.pop((s, m, j))
                        for hh in range(2):
                            nc.tensor.matmul(
                                av[:, hh, :],
                                V_sb[2 * m + hh][:, j, :],
                                ex[:, hh, :],
                                start=(j == 0),
                                stop=(j == KT - 1),
                            )
                        if j == KT - 1:
                            av = avs.pop((s, m))
                            norm_group(av, m, s)

        if dbg is not None:
            for s in range(NSLAB):
                nc.sync.dma_start(out=dbg["dbg_qt"][:, :, WQ * s:WQ * (s + 1)], in_=QT_sb[s])
                nc.sync.dma_start(out=dbg["dbg_ot"][:, :, WQ * s:WQ * (s + 1)], in_=OT_sb[s])
            nc.sync.dma_start(out=dbg["dbg_kt"], in_=KT_sb)
            for h in range(HG):
                nc.sync.dma_start(out=dbg["dbg_v"][:, :, h, :], in_=V_sb[h])

        # ---- stage 3: output projection tail; bf16 partials ----
        with nc.named_scope("outproj"):
            with tc.tile_pool(name="psFO", bufs=8, space="PSUM") as psFO:
                nfo = [0]
                for t in range(S // 128):
                    ob = outp.tile([128, D], BF16, tag="ob", name="ob")
                    for half in range(2):
                        fo = psFO.tile([128, 512], F32, tag="fo", name="fo")
                        for m in range(MT):
                            nc.tensor.matmul(
                                fo,
                                OT_sb[t // 4][:, m, 128 * (t % 4):128 * (t % 4 + 1)],
                                wo_sb[:, m, 512 * half:512 * (half + 1)],
                                start=(m == 0),
                                stop=(m == MT - 1),
                            )
                        dst = ob[:, 512 * half:512 * (half + 1)]
                        # DVE's f32->bf16 CAST (~376ns) is ~2x faster than
                        # ACT's copy; weight the split 2:1 toward DVE
                        if nfo[0] % 3 == 2:
                            nc.scalar.activation(
                                dst, fo, mybir.ActivationFunctionType.Copy
                            )
                        else:
                            nc.vector.tensor_copy(dst, fo)
                        nfo[0] += 1
                        nc.sync.dma_start(
                            out=out[128 * t:128 * (t + 1),
                                    512 * half:512 * (half + 1)],
                            in_=dst,
                        )


def _get_nc():
    if "nc" not in _CACHE:
        _CACHE["nc"] = _build_nc()
    return _CACHE["nc"]


def make_in_maps(query, key, value, Wq, bq, Wk, bk, Wv, bv, Wo, bo):
    f32 = lambda a: np.asarray(a, dtype=np.float32)
    f16 = lambda a: np.ascontiguousarray(np.asarray(a, dtype=np.float32).astype(NPDT))
    query, key, value = f32(query), f32(key), f32(value)
    bq, bk = np.ascontiguousarray(f32(bq)), np.ascontiguousarray(f32(bk))
    Wq, Wk, Wv, Wo = f32(Wq), f32(Wk), f32(Wv), f32(Wo)

    xT = [[f16(x[b].T) for b in range(B)] for x in (query, key, value)]
    in_maps = []
    for c in range(NCORES):
        b, g = c // GROUPS, c % GROUPS
        sl = slice(IS * g, IS * (g + 1))
        in_maps.append({
            "xqT": xT[0][b],
            "xkT": xT[1][b],
            "xvT": xT[2][b],
            "wq": f16(Wq[:, sl]),
            "wk": f16(Wk[:, sl]),
            "wv": f16(Wv[:, sl]),
            "wo": f16(Wo[sl, :]),
            "bq": np.ascontiguousarray(bq[sl]),
            "bk": np.ascontiguousarray(bk[sl]),
        })
    return in_maps


def combine_outputs(results, bv, Wo, bo):
    bo = np.asarray(bo, dtype=np.float32)
    bv = np.asarray(bv, dtype=np.float32)
    Wo = np.asarray(Wo, dtype=np.float32)
    bias = bo + bv @ Wo
    out = np.empty((B, S, D), dtype=np.float32)
    for b in range(B):
        acc = results[b * GROUPS]["out"].astype(np.float32)
        for g in range(1, GROUPS):
            acc = acc + results[b * GROUPS + g]["out"].astype(np.float32)
        out[b] = acc + bias
    return out


def kernel(query, key, value, Wq, bq, Wk, bk, Wv, bv, Wo, bo):
    nc = _get_nc()
    in_maps = make_in_maps(query, key, value, Wq, bq, Wk, bk, Wv, bv, Wo, bo)
    try:
        res = run_bass_kernel_spmd(nc, in_maps, list(range(NCORES)))
    except Exception:
        # a fresh NEFF's first execution occasionally reports
        # NRT_EXEC_UNIT_UNRECOVERABLE; a retry reliably succeeds
        res = run_bass_kernel_spmd(nc, in_maps, list(range(NCORES)))
    return combine_outputs(res.results, bv, Wo, bo)


# revision 20
# speedup vs baseline: 1.4200x; 1.0468x over previous
"""Multi-head attention (B=2, S=2048, D=1024, 16 heads x 64) on 8 NeuronCores.

Sharding: batch x head-group data/tensor parallel. Core c handles batch
c//4 and heads [4*(c%4), 4*(c%4)+4). Wq/Wk/Wv are column-sliced per head
group, Wo row-sliced; each core emits a partial [S, D] output (bf16) and
the host sums the 4 partials per batch and adds bo + bv@Wo (the V bias
commutes through attention since softmax rows sum to 1, so it is folded
into the final bias host-side).

v2 over the first working kernel (243.6us):
  1. Pair-packed Q^T/K^T: heads 2m / 2m+1 live in partitions 0:64 /
     64:128 of one [128, S] tile. The two heads' score matmuls (K=64
     contraction each) then run CONCURRENTLY in disjoint 32-row groups
     of the PE array (tile_position auto-derived from base_partition),
     halving scores PE time.
  2. exp split across ACT and a custom DVE op pair: P1 = deg-4 Horner
     q ~= exp(s/32)/c0 (8 ALU stages; coeffs via s0/s1/imm2 + C3 spill;
     constant c0 normalized away - softmax is scale-invariant, and the
     ACT path matches the scale via exp bias = -4*ln(c0)); P2 = q^4 via
     two squarings. ~0.01% poly error. ACT alone was the 136us phase
     bottleneck; split, both engines stay under the PE's attention time.
  3. V projected directly into natural [seq, dh] layout (stationary =
     x^T seq-chunk, moving = Wv), killing the PE transposes. Per-head
     128-col stationary blocks hold [V|1|0] (even head) / [1|0|V] (odd
     head) so attnV emits O^T rows partition-aligned for both heads and
     row 64/0 of the accumulator is the softmax denominator.
  4. Garbage-operand warmup matmuls at t=0 (no DMA deps) open the HAM
     clock gate (~3.4us busy window) before the real projections start;
     the whole PE stream is then gap-free so the gate never re-closes.
  5. Output projection as a short tail phase (PSUM too tight to drip it
     through attention); partials DMA'd as bf16 to halve the tail DMA.
"""

import sys

if "/opt/trn_rl_repo" not in sys.path:
    sys.path.insert(0, "/opt/trn_rl_repo")

import numpy as np

import concourse.bacc as bacc
import concourse.mybir as mybir
import concourse.tile as tile
from concourse.bass_utils import run_bass_kernel_spmd

# ---- custom DVE exp ops (registered at import; appended to the op table
# so existing rows keep their opcodes) --------------------------------------
import concourse.dve_ops as dve_ops
from concourse.dve_ops import DveOp
from concourse.dve_spec import (
    C0, C1, C2, One, Src0, Spec, sq, _has_src1, lower,
)
from concourse.dve_uop import DveOpSpec
from concourse.dve_table_gen import dve_ver_for

# cubic weighted-minimax fit of exp(s/32), normalized so the constant
# term is 1 (true constant c0 below; softmax cancels it, ACT compensates).
# One DVE instruction computes exp(s/8)/c0^4: Horner cubic (6 stages) +
# two squarings (2 stages) = the full 8-stage v3 budget.
D3, D2, D1 = (5.360408300406049e-06, 0.0005106809430822295,
              0.03125779746235493)
C0_NORM = 0.9988460037558242
ACT_BIAS = float(-4.0 * np.log(C0_NORM))  # exp(s/8 + bias) = exp(s/8)/c0^4


def _expc_ref(in0, in1, s0, s1, imm2):
    x = in0.astype(np.float32)
    q = ((s0 * x + s1) * x + imm2) * x + 1.0
    return (q * q) * (q * q)


def _register(name, body, ref):
    for op in dve_ops.OPS:
        if op.name == name:
            return op
    spec = Spec(body=body, reference=ref)
    row = dve_ops._CUSTOM_DVE_ROW_BASE + len(dve_ops.OPS)
    ver = dve_ver_for("TRN2")
    sha = DveOpSpec(
        name=name, opcode=row, uops=lower(spec, ver=ver),
        rd1_en=_has_src1(spec),
    ).sha(ver)
    op = DveOp(name, spec, subdim=False, uops_sha={ver: sha})
    dve_ops.OPS.append(op)
    dve_ops._SUB_OPCODE_FOR_NAME[name] = row
    dve_ops.CUSTOM_DVE_SPECS[name] = spec
    return op


EXP_C34 = _register(
    "EXP_C34_ANT",
    sq(sq((((C0 * Src0 + C1) * Src0 + C2) * Src0) + One)),
    _expc_ref,
)

F32 = mybir.dt.float32
F16 = mybir.dt.float16
BF16 = mybir.dt.bfloat16
NPDT = np.float16

B, S, D = 2, 2048, 1024
NH, DH = 16, 64
NCORES = 8
GROUPS = 4                # head-groups (cores per batch)
HG = NH // GROUPS         # heads per core = 4
IS = HG * DH              # inner slice per core = 256
KD = D // 128             # contraction chunks for projections = 8
MT = IS // 128            # head-pairs per core = 2
KT = S // 128             # 128-row key chunks = 16
WQ = 512                  # q-slab width
NSLAB = S // WQ           # q slabs = 4
NWARM = 16                # garbage warmup matmuls
ACT_JOBS = 74             # exp jobs on ACT (rest on the DVE custom op)

_CACHE = {}


def _build_nc():
    nc = bacc.Bacc("TRN2", target_bir_lowering=False, debug=False)

    xqT = nc.dram_tensor("xqT", [D, S], F16, kind="ExternalInput").ap()
    xkT = nc.dram_tensor("xkT", [D, S], F16, kind="ExternalInput").ap()
    xvT = nc.dram_tensor("xvT", [D, S], F16, kind="ExternalInput").ap()
    wq = nc.dram_tensor("wq", [D, IS], F16, kind="ExternalInput").ap()
    wk = nc.dram_tensor("wk", [D, IS], F16, kind="ExternalInput").ap()
    wv = nc.dram_tensor("wv", [D, IS], F16, kind="ExternalInput").ap()
    wo = nc.dram_tensor("wo", [IS, D], F16, kind="ExternalInput").ap()
    bq = nc.dram_tensor("bq", [IS], F32, kind="ExternalInput").ap()
    bk = nc.dram_tensor("bk", [IS], F32, kind="ExternalInput").ap()
    out = nc.dram_tensor("out", [D, S], BF16, kind="ExternalOutput").ap()

    import os
    dbg = None
    if os.environ.get("MHA_DEBUG"):
        dbg = {
            "dbg_qt": nc.dram_tensor("dbg_qt", [128, MT, S], F16, kind="ExternalOutput").ap(),
            "dbg_kt": nc.dram_tensor("dbg_kt", [128, MT, S], F16, kind="ExternalOutput").ap(),
            "dbg_v": nc.dram_tensor("dbg_v", [128, KT, HG, 128], F16, kind="ExternalOutput").ap(),
            "dbg_ot": nc.dram_tensor("dbg_ot", [128, MT, S], F16, kind="ExternalOutput").ap(),
        }

    with tile.TileContext(nc) as tc:
        _emit(nc, tc, xqT, xkT, xvT, wq, wk, wv, wo, bq, bk, out, dbg)
    nc.compile()
    return nc


def _emit(nc, tc, xqT, xkT, xvT, wq, wk, wv, wo, bq, bk, out, dbg=None):
    from contextlib import ExitStack

    ctx = ExitStack()
    with ctx:
        consts = ctx.enter_context(tc.tile_pool(name="consts", bufs=1))
        big = ctx.enter_context(tc.tile_pool(name="big", bufs=1))
        xin = ctx.enter_context(tc.tile_pool(name="xin", bufs=4))
        expp = ctx.enter_context(tc.tile_pool(name="expp", bufs=6))
        smallp = ctx.enter_context(tc.tile_pool(name="smallp", bufs=4))
        bcsp = ctx.enter_context(tc.tile_pool(name="bcsp", bufs=2))
        outp = ctx.enter_context(tc.tile_pool(name="outp", bufs=4))

        # ---- constants ----
        col1_f = consts.tile([128, 1], F16, name="col1_f")
        nc.vector.memset(col1_f, 1.0)
        abias = consts.tile([128, 1], F32, name="abias")
        nc.vector.memset(abias, ACT_BIAS)
        garb_w = consts.tile([128, 128], F16, name="garb_w")
        garb_x = consts.tile([128, 512], F16, name="garb_x")
        nc.vector.memset(garb_w, 0.0)
        nc.vector.memset(garb_x, 0.0)
        dum = consts.tile([1, 2], F32, name="dum")
        nc.vector.memset(dum, 0.0)
        dum16 = consts.tile([1, 2], F16, name="dum16")
        wq_sb = consts.tile([128, KD, IS], F16, name="wq_sb")
        wk_sb = consts.tile([128, KD, IS], F16, name="wk_sb")
        wv_sb = consts.tile([128, KD, IS], F16, name="wv_sb")
        wo_sb = consts.tile([128, MT, D], F16, name="wo_sb")
        bq_sb = consts.tile([128, MT], F32, name="bq_sb")
        bk_sb = consts.tile([128, MT], F32, name="bk_sb")

        # ---- persistent intermediates ----
        # QT/KT pair-packed: partitions 0:64 = head 2m, 64:128 = head 2m+1
        QT_sb = [
            big.tile([128, MT, WQ], F16, name=f"QT_sb{s}") for s in range(NSLAB)
        ]
        KT_sb = big.tile([128, MT, S], F16, name="KT_sb")
        VT_sb = big.tile([128, MT, S], F16, name="VT_sb")
        # V natural per (chunk, head): [V(64)|1|0...] even, [1|0...|V(64)] odd
        V_sb = [
            big.tile([128, KT, 128], F16, name=f"V_sb{h}") for h in range(HG)
        ]
        OT_sb = [
            big.tile([128, MT, WQ], F16, name=f"OT_sb{s}") for s in range(NSLAB)
        ]
        for h in range(HG):
            nc.gpsimd.memset(V_sb[h], 0.0)
            c = DH if h % 2 == 0 else 0
            nc.vector.tensor_copy(
                V_sb[h][:, :, c:c + 1],
                col1_f.unsqueeze(1).broadcast_to([128, KT, 1]),
            )

        # trigger the ACT exp table DMA (~2.7us) during the initial input
        # DMAs instead of at the first real exp
        nc.scalar.activation(dum16, dum, mybir.ActivationFunctionType.Exp)

        # PSUM->SBUF moves alternating between DVE and ACT
        _eng = [0]

        def bias_copy(dst_ap, src_ap, bias_ap):
            if _eng[0] % 2 == 0:
                nc.vector.tensor_scalar_add(dst_ap, src_ap, bias_ap)
            else:
                nc.scalar.activation(
                    dst_ap, src_ap,
                    mybir.ActivationFunctionType.Identity,
                    bias=bias_ap,
                )
            _eng[0] += 1

        def plain_copy(dst_ap, src_ap):
            if _eng[0] % 2 == 0:
                nc.vector.tensor_copy(dst_ap, src_ap)
            else:
                nc.scalar.activation(
                    dst_ap, src_ap, mybir.ActivationFunctionType.Copy
                )
            _eng[0] += 1

        # ---- stage 1a: Q/K projections (transposed layout, pair-packed) ----
        NS2 = S // 512
        with nc.named_scope("proj"):
            with tc.tile_pool(name="psQK", bufs=8, space="PSUM") as psQK:
                # HAM warmup: garbage matmuls with no DMA dependencies run
                # immediately; ~3.4us of cold PE activity opens the clock
                # gate before the first real matmul needs it.
                for i in range(NWARM):
                    wt = psQK.tile([128, 512], F32, tag="ps", name=f"warm{i}")
                    nc.tensor.matmul(wt, garb_w, garb_x, start=True, stop=True)
                for xT, w_dram, w_sb, b_dram, b_sb, dest in (
                    (xkT, wk, wk_sb, bk, bk_sb, KT_sb),
                    (xqT, wq, wq_sb, bq, bq_sb, QT_sb),
                    (xvT, wv, wv_sb, None, None, VT_sb),
                ):
                    nc.sync.dma_start(
                        out=w_sb, in_=w_dram.rearrange("(k p) i -> p k i", p=128)
                    )
                    if b_dram is not None:
                        nc.sync.dma_start(
                            out=b_sb, in_=b_dram.rearrange("(m p) -> p m", p=128)
                        )
                    ps = [
                        [
                            psQK.tile([128, 512], F32, tag="ps", name=f"ps_{m}_{n}")
                            for n in range(NS2)
                        ]
                        for m in range(MT)
                    ]
                    for k in range(KD):
                        xt = xin.tile([128, S], F16, tag="xt")
                        nc.sync.dma_start(out=xt, in_=xT[128 * k:128 * (k + 1), :])
                        for m in range(MT):
                            for n in range(NS2):
                                nc.tensor.matmul(
                                    ps[m][n],
                                    w_sb[:, k, 128 * m:128 * (m + 1)],
                                    xt[:, 512 * n:512 * (n + 1)],
                                    start=(k == 0),
                                    stop=(k == KD - 1),
                                )
                    for m in range(MT):
                        for n in range(NS2):
                            if dest is QT_sb:
                                dst = QT_sb[n][:, m, :]
                            else:
                                dst = dest[:, m, 512 * n:512 * (n + 1)]
                            if b_sb is None:
                                plain_copy(dst, ps[m][n])
                            else:
                                bias_copy(dst, ps[m][n], b_sb[:, m:m + 1])
                nc.sync.dma_start(
                    out=wo_sb, in_=wo.rearrange("(c p) d -> p c d", p=128)
                )
            # warm-bridge: keep the PE busy through the psQK->psAT pool
            # drain (V^T copies) so the HAM clock gate stays open into
            # the attention phase
                for i in range(6):
                    wt = psQK.tile([128, 512], F32, tag="ps", name=f"wb{i}")
                    nc.tensor.matmul(wt, garb_w, garb_x, start=True, stop=True)
            # V^T -> V natural via DMA-transpose XBAR (off the PE and off
            # ACT/DVE): one [64, S] -> [S(=128x16), 64] transpose per head,
            # written straight into the strided per-head V blocks.
            for h in range(HG):
                c = 0 if h % 2 == 0 else DH
                nc.sync.dma_start(
                    out=V_sb[h][:, :, c:c + DH],
                    in_=VT_sb[64 * (h % 2):64 * (h % 2) + DH, h // 2, :],
                    transpose=True,
                )

            # ---- stage 1b: V projection straight into natural layout ----
            # stationary = x^T seq-chunk, moving = Wv chunk; all 16 seq
            # tiles accumulate in PSUM at once (16 x 1KB = 8 banks).
        # ---- stage 2: attention, software-pipelined over (slab, pair, j).
        # Per iter: 2 concurrent score MMs -> one exp job (ACT or DVE
        # custom-op pair, alternating) -> 2 attnV MMs two iters later. ----
        with nc.named_scope("attn"):
            with tc.tile_pool(name="psAT", bufs=2, space="PSUM") as psAT:
                NIT = NSLAB * MT * KT
                LAG = 2
                wslot = psAT.tile([128, 2, 512], F32, tag="av", name="wbridge")
                exs = {}
                avs = {}
                _exp_eng = [0]

                def norm_group(av, m, s):
                    # denominator rows: 64 (even head) / 0 (odd head)
                    for hh in range(2):
                        p0 = 0 if hh == 0 else DH
                        srow = DH if hh == 0 else 0
                        sums = smallp.tile([1, WQ], F32, tag="sums")
                        nc.scalar.activation(
                            sums, av[srow:srow + 1, hh, :],
                            mybir.ActivationFunctionType.Copy,
                        )
                        rec = smallp.tile([1, WQ], F32, tag="rec")
                        nc.vector.reciprocal_approx_fast(rec, sums)
                        bcs = bcsp.tile([DH, WQ], F32, tag="bcs")
                        nc.gpsimd.partition_broadcast(bcs, rec)
                        nc.vector.tensor_mul(
                            OT_sb[s][p0:p0 + DH, m, :],
                            av[p0:p0 + DH, hh, :],
                            bcs,
                        )

                for idx in range(NIT + LAG):
                    if idx < NIT:
                        s, mj = divmod(idx, MT * KT)
                        m, j = divmod(mj, KT)
                        sc = psAT.tile([128, 2, 512], F32, tag="sc", name="sc")
                        for hh in range(2):
                            p0 = 64 * hh
                            nc.tensor.matmul(
                                sc[:, hh, :],
                                KT_sb[p0:p0 + DH, m, 128 * j:128 * (j + 1)],
                                QT_sb[s][p0:p0 + DH, m, :],
                                start=True, stop=True,
                            )
                        ex = expp.tile([128, 2, 512], F16, tag="ex")
                        i_e = _exp_eng[0]
                        on_act = (i_e * ACT_JOBS) // NIT != ((i_e + 1) * ACT_JOBS) // NIT
                        if on_act:
                            nc.scalar.activation(
                                ex, sc, mybir.ActivationFunctionType.Exp,
                                scale=0.125, bias=abias,
                            )
                        else:
                            nc.vector._custom_dve(
                                EXP_C34, out=ex, in0=sc,
                                s0=D3, s1=D2, imm2=D1,
                            )
                        _exp_eng[0] += 1
                        exs[(s, m, j)] = ex
                    if idx < 4:
                        # HAM bridge across the exp-pipeline fill
                        for _ in range(2):
                            nc.tensor.matmul(
                                wslot[:, 0, :], garb_w, garb_x,
                                start=True, stop=True,
                            )
                    if idx >= LAG:
                        s, mj = divmod(idx - LAG, MT * KT)
                        m, j = divmod(mj, KT)
                        if j == 0:
                            avs[(s, m)] = psAT.tile(
                                [128, 2, 512], F32, tag="av", name="av"
                            )
                        av = avs[(s, m)]
                        ex = exs.pop((s, m, j))
                        for hh in range(2):
                            nc.tensor.matmul(
                                av[:, hh, :],
                                V_sb[2 * m + hh][:, j, :],
                                ex[:, hh, :],
                                start=(j == 0),
                                stop=(j == KT - 1),
                            )
                        if j == KT - 1:
                            av = avs.pop((s, m))
                            norm_group(av, m, s)

                # ---- output projection tail (inside psAT: fo tiles
                # reuse the attention slots, alternating sc/av tags - a
                # fresh pool would drain-barrier on the final norm reads,
                # idling the PE ~7us and dropping the HAM clock; sc slots
                # free at the last exp, so outproj starts immediately).
                # Computed TRANSPOSED (stationary = Wo chunks, moving =
                # O^T slabs): 16 reused stationaries instead of 64, and
                # the host transposes the [D, S] partial for free. ----
                with nc.named_scope("outproj"):
                    nfo = 0
                    for c in range(KD):
                        ob = outp.tile([128, S], BF16, tag="ob", name="ob")
                        for s in range(NSLAB):
                            slot = psAT.tile(
                                [128, 2, 512], F32,
                                tag=("sc", "av")[nfo % 2], name="fo",
                            )
                            fo = slot[:, 0, :]
                            for m in range(MT):
                                nc.tensor.matmul(
                                    fo,
                                    wo_sb[:, m, 128 * c:128 * (c + 1)],
                                    OT_sb[s][:, m, :],
                                    start=(m == 0),
                                    stop=(m == MT - 1),
                                )
                            dst = ob[:, 512 * s:512 * (s + 1)]
                            # DVE's f32->bf16 CAST is faster than ACT's copy;
                            # weight the split toward DVE
                            if nfo % 3 == 2:
                                nc.scalar.activation(
                                    dst, fo, mybir.ActivationFunctionType.Copy
                                )
                            else:
                                nc.vector.tensor_copy(dst, fo)
                            nfo += 1
                        # one contiguous [128, S] DMA per chunk: 8 big DMAs
                        # instead of 32 small triggers on the sync queue
                        nc.sync.dma_start(out=out[128 * c:128 * (c + 1), :], in_=ob)

        if dbg is not None:
            for s in range(NSLAB):
                nc.sync.dma_start(out=dbg["dbg_qt"][:, :, WQ * s:WQ * (s + 1)], in_=QT_sb[s])
                nc.sync.dma_start(out=dbg["dbg_ot"][:, :, WQ * s:WQ * (s + 1)], in_=OT_sb[s])
            nc.sync.dma_start(out=dbg["dbg_kt"], in_=KT_sb)
            for h in range(HG):
                nc.sync.dma_start(out=dbg["dbg_v"][:, :, h, :], in_=V_sb[h])
